# revision 1
# baseline (speedup 1.0000x reference)
"""Causal multi-head attention kernel for Trainium2 (8 NeuronCores).

Problem: B=2, H=16, S=2048, D=64 causal attention (softmax over last axis).
Sharding: 32 (batch, head) pairs split 4-per-core across 8 cores; each core
computes its heads independently (no collectives).

Per-core algorithm (all in the "S-transposed" layout so no transposes of the
probability matrix are ever needed):
  - Host pre-packs, per core:
      qt  [2, 128, 2048] bf16 : two head-PAIRS; partitions 0-63 = head 2p's
                                Q^T (d-major), partitions 64-127 = head 2p+1's
      kt  [2, 128, 2048] bf16 : same for K
      vg  [4, 128, 16, 65] bf16: per head, V tiles [128, 65] with a ones
                                column appended (col 64) -> PV matmul also
                                produces the softmax row-sum for free
      tri [128, 128] bf16     : tri[k, c] = 1 if c >= k else 0 (intra-block
                                causal keep-mask in S^T coords)
  - For each head, for each k-block kb (128 keys):
      S^T strip [k=128, q=kb*128..2047] = K_blk^T.T @ Q^T  (PE, bf16)
      P^T = exp(S^T / 8)               (ACT, PSUM->SBUF, bf16 out)
      diagonal 128x128 block masked via elementwise tri multiply (DVE)
      O accumulation: for each 128-row q block, acc[q] += P^T_chunk.T @ V_blk
        (PE, accumulating in PSUM; 16 accumulators packed 7/7/2 per bank,
         one has_written "zero region" start per bank)
  - Normalize: O[q, :64] * (1 / O[q, 64]) (DVE reciprocal + per-partition
    scalar multiply), DMA out as f32.

kernel(Q, K, V, mask) takes the full unsharded fp32 inputs and returns the
full [2, 16, 2048, 64] fp32 output.
"""

import sys

if "/opt/trn_rl_repo" not in sys.path:
    sys.path.insert(0, "/opt/trn_rl_repo")

import numpy as np
import ml_dtypes

B, H, S, D = 2, 16, 2048, 64
N_CORES = 8
HEADS_PER_CORE = (B * H) // N_CORES  # 4
KB = S // 128  # 16 k-blocks per head
QS = S // 128  # 16 q-subblocks per head

_BF16 = ml_dtypes.bfloat16

# accumulator bank packing: q_subs 0-6 -> bank A, 7-13 -> bank B, 14-15 -> C
_BANK_FIRST = (0, 7, 14)   # first q_sub written in each accumulator bank
_BANK_LAST = (6, 13, 15)   # last q_sub written in each accumulator bank

_built = {}
STRIP_W = 1024
MODE = "full"  # full | qk_only | qk_exp (timing ablations)
ST_BUFS = 2  # PSUM strip-tile slots  # S^T strip tile width (PSUM free elems)


def _emit(tc, nc, mybir, qt, kt, vg, tri, o, causal, reps=1):
    from contextlib import ExitStack

    f32 = mybir.dt.float32
    bf = mybir.dt.bfloat16
    Exp = mybir.ActivationFunctionType.Exp

    with ExitStack() as ctx:
        const = ctx.enter_context(tc.tile_pool(name="const", bufs=1))
        qk = ctx.enter_context(tc.tile_pool(name="qk", bufs=2))
        vpool = ctx.enter_context(tc.tile_pool(name="vp", bufs=2))
        ptp = ctx.enter_context(tc.tile_pool(name="ptp", bufs=4))
        outp = ctx.enter_context(tc.tile_pool(name="outp", bufs=4))
        small = ctx.enter_context(tc.tile_pool(name="small", bufs=4))
        stp = ctx.enter_context(tc.tile_pool(name="stp", bufs=ST_BUFS, space="PSUM"))
        accp = ctx.enter_context(tc.tile_pool(name="accp", bufs=1, space="PSUM"))

        tri_t = const.tile([128, 128], bf, name="tri_t")
        nc.sync.dma_start(tri_t[:, :], tri[:, :])

        # dummy exp issued first: walrus places the ~2.7us ACT table load
        # before the first ACTIVATE in the stream, so doing one on a tiny
        # constant tile overlaps the table load with the input DMAs instead
        # of serializing it before the first real exp
        warm = const.tile([128, 1], f32, name="warm")
        nc.vector.memset(warm[:, :], 0.0)
        nc.scalar.activation(warm[:, :], warm[:, :], Exp)

        from contextlib import nullcontext
        with (tc.For_i(0, reps, 1) if reps > 1 else nullcontext()):
          rep = 0  # body emitted once; hardware loop repeats it
          for p in range(HEADS_PER_CORE // 2):
              # qt via SP queue, kt via DVE queue so the two big loads overlap;
              # chunk them so the first QK matmul can start early.
              qt_t = qk.tile([128, S], bf, tag="qt", name=f"qt_{rep}_{p}")
              kt_t = qk.tile([128, S], bf, tag="kt", name=f"kt_{rep}_{p}")
              # kt on the gpsimd SWDGE queue, qt on the SP HWDGE queue so they
              # load in parallel; the leading chunks unblock the first QK early.
              nc.gpsimd.dma_start(kt_t[:, :128], kt[p][:, :128])
              nc.sync.dma_start(qt_t[:, :512], qt[p][:, :512])
              nc.sync.dma_start(qt_t[:, 512:1024], qt[p][:, 512:1024])
              nc.gpsimd.dma_start(kt_t[:, 128:], kt[p][:, 128:])
              nc.sync.dma_start(qt_t[:, 1024:], qt[p][:, 1024:])
              for s2 in range(2):
                  h = 2 * p + s2
                  po = 64 * s2  # partition offset of this head's d-dim
                  vg_t = vpool.tile([128, KB, 65], bf, tag="vg", name=f"vg_{rep}_{h}")
                  nc.gpsimd.dma_start(vg_t[:, :, :], vg[h])

                  accA = accp.tile([128, 7, 65], f32, tag="accA", name=f"accA_{rep}_{h}")
                  accB = accp.tile([128, 7, 65], f32, tag="accB", name=f"accB_{rep}_{h}")
                  accC = accp.tile([128, 2, 65], f32, tag="accC", name=f"accC_{rep}_{h}")

                  def acc(i):
                      if i < 7:
                          return accA[:, i, :]
                      if i < 14:
                          return accB[:, i - 7, :]
                      return accC[:, i - 14, :]

                  def strip_halves(kb):
                      q0 = 128 * kb if causal else 0
                      cols = S - q0
                      pieces = []
                      hs = 0
                      while hs < cols:
                          pieces.append((q0, hs, min(1024, cols - hs), "A"))
                          hs += 1024
                      return pieces

                  def emit_qk(kb):
                      """QK matmuls for strip kb; returns the st tiles."""
                      sts = []
                      for q0, hs, hw, tg in strip_halves(kb):
                          st = stp.tile([128, 1024], f32, tag="st",
                                        name=f"st_{rep}_{h}_{kb}_{hs}")
                          for c0 in range(0, hw, 512):
                              cw = min(512, hw - c0)
                              nc.tensor.matmul(
                                  st[:, c0:c0 + cw],
                                  lhsT=kt_t[po:po + 64, kb * 128:(kb + 1) * 128],
                                  rhs=qt_t[po:po + 64,
                                           q0 + hs + c0:q0 + hs + c0 + cw],
                                  start=True, stop=True,
                              )
                          sts.append(st)
                      return sts

                  def normalize_bank(qs_lo, qs_hi):
                      """Normalize q_subs [qs_lo, qs_hi) and write out in one
                      batched DMA (rows r of out tile j map to q = qs*128+r)."""
                      n = qs_hi - qs_lo
                      ot = outp.tile([128, n, 64], f32, tag=f"ot{qs_lo}",
                                     name=f"ot_{rep}_{h}_{qs_lo}")
                      for j in range(n):
                          a = acc(qs_lo + j)
                          rs = small.tile([128, 1], f32, tag="rs",
                                          name=f"rs_{rep}_{h}_{qs_lo + j}")
                          nc.vector.reciprocal(rs[:, :], a[:, 64:65])
                          nc.vector.tensor_scalar_mul(ot[:, j, :], a[:, :64],
                                                      rs[:, :])
                      dst = o[h, qs_lo * 128:qs_hi * 128, :].rearrange(
                          "(j r) c -> r j c", r=128)
                      nc.sync.dma_start(dst, ot[:, :, :])

                  sts = emit_qk(0)
                  for kb in range(KB):
                      # exp of strip kb
                      pts = []
                      for (q0, hs, hw, tg), st in zip(strip_halves(kb), sts):
                          if MODE == "qk_only":
                              continue
                          pt = ptp.tile([128, 1024], bf, tag="pt",
                                        name=f"pt_{rep}_{h}_{kb}_{hs}")
                          nc.scalar.activation(pt[:, :hw], st[:, :hw], Exp,
                                               scale=0.125)
                          if causal and hs == 0:
                              nc.vector.tensor_mul(pt[:, :128], pt[:, :128],
                                                   tri_t[:, :])
                          pts.append(pt)
                      # QK for strip kb+1 goes to PE before PV of strip kb so
                      # the PE never stalls behind ACT
                      if kb + 1 < KB:
                          sts = emit_qk(kb + 1)
                      # PV accumulation for strip kb
                      if MODE != "full":
                          continue
                      for (q0, hs, hw, tg), pt in zip(strip_halves(kb), pts):
                          qs_range = list(range((q0 + hs) // 128,
                                                (q0 + hs + hw) // 128))
                          # the diagonal q_sub (== kb) additionally depends on
                          # the DVE tri-multiply; emit it last so the PE can
                          # start the other PV matmuls as soon as exp is done.
                          # (at kb==0 keep ascending order: the bank-group
                          # start=True matmuls must be first into each bank)
                          if causal and kb > 0 and qs_range and qs_range[0] == kb:
                              qs_range = qs_range[1:] + [kb]
                          for q_sub in qs_range:
                              m = q_sub * 128 - q0 - hs
                              last_kb = q_sub if causal else KB - 1
                              nc.tensor.matmul(
                                  acc(q_sub),
                                  lhsT=pt[:, m:m + 128],
                                  rhs=vg_t[:, kb, :],
                                  start=(kb == 0 and q_sub in _BANK_FIRST),
                                  stop=(q_sub in _BANK_LAST and kb == last_kb),
                              )
                      # normalize accumulator banks as soon as they complete
                      if causal:
                          if kb == 6:
                              normalize_bank(0, 7)
                          elif kb == 13:
                              normalize_bank(7, 14)
                          elif kb == 15:
                              normalize_bank(14, 16)
                  if not causal:
                      normalize_bank(0, 7)
                      normalize_bank(7, 14)
                      normalize_bank(14, 16)


def build_nc(causal=True, reps=1):
    """Build + compile the per-core Bass program (cached)."""
    key = ("nc", causal, reps, STRIP_W, MODE, ST_BUFS)
    if key in _built:
        return _built[key]
    import concourse.bacc as bacc
    from concourse import mybir, tile

    nc = bacc.Bacc("TRN2", target_bir_lowering=False, debug=False,
                   num_devices=N_CORES)
    qt = nc.dram_tensor("qt", (HEADS_PER_CORE // 2, 128, S),
                        mybir.dt.bfloat16, kind="ExternalInput").ap()
    kt = nc.dram_tensor("kt", (HEADS_PER_CORE // 2, 128, S),
                        mybir.dt.bfloat16, kind="ExternalInput").ap()
    vg = nc.dram_tensor("vg", (HEADS_PER_CORE, 128, KB, 65),
                        mybir.dt.bfloat16, kind="ExternalInput").ap()
    tri = nc.dram_tensor("tri", (128, 128), mybir.dt.bfloat16,
                         kind="ExternalInput").ap()
    o = nc.dram_tensor("o", (HEADS_PER_CORE, S, D), mybir.dt.float32,
                       kind="ExternalOutput").ap()
    with tile.TileContext(nc) as tc:
        _emit(tc, nc, mybir, qt, kt, vg, tri, o, causal, reps)
    nc.compile()
    _built[key] = nc
    return nc


def prep_inputs(Q, K, V):
    """Host-side shard + layout prep. Returns list of 8 per-core input dicts."""
    Qf = np.ascontiguousarray(Q, dtype=np.float32).reshape(B * H, S, D)
    Kf = np.ascontiguousarray(K, dtype=np.float32).reshape(B * H, S, D)
    Vf = np.ascontiguousarray(V, dtype=np.float32).reshape(B * H, S, D)

    # [BH, S, D] -> transposed, bf16: [BH, D, S]
    Qt = np.ascontiguousarray(Qf.transpose(0, 2, 1)).astype(_BF16)
    Kt = np.ascontiguousarray(Kf.transpose(0, 2, 1)).astype(_BF16)

    # V augmented with ones column, partition-major: [BH, 128, KB, 65]
    Vb = Vf.astype(_BF16)
    vg_all = np.empty((B * H, 128, KB, 65), dtype=_BF16)
    # V[h, kb*128 + r, c] -> vg[h, r, kb, c]
    vg_all[:, :, :, :64] = Vb.reshape(B * H, KB, 128, D).transpose(0, 2, 1, 3)
    vg_all[:, :, :, 64] = _BF16(1.0)

    tri_np = (np.tril(np.ones((128, 128), dtype=np.float32))
              .T.astype(_BF16))  # tri[k, c] = 1 if c >= k
    tri_np = np.ascontiguousarray(tri_np)

    in_maps = []
    for c in range(N_CORES):
        h0 = c * HEADS_PER_CORE
        qt_c = np.empty((HEADS_PER_CORE // 2, 128, S), dtype=_BF16)
        kt_c = np.empty((HEADS_PER_CORE // 2, 128, S), dtype=_BF16)
        for p in range(HEADS_PER_CORE // 2):
            qt_c[p, :64] = Qt[h0 + 2 * p]
            qt_c[p, 64:] = Qt[h0 + 2 * p + 1]
            kt_c[p, :64] = Kt[h0 + 2 * p]
            kt_c[p, 64:] = Kt[h0 + 2 * p + 1]
        in_maps.append({
            "qt": qt_c,
            "kt": kt_c,
            "vg": np.ascontiguousarray(vg_all[h0:h0 + HEADS_PER_CORE]),
            "tri": tri_np,
        })
    return in_maps


def _classify_mask(mask):
    m = np.asarray(mask).reshape(S, S)
    if not m.any():
        return "dense"
    if np.array_equal(m, np.triu(np.ones((S, S), dtype=bool), k=1)):
        return "causal"
    raise NotImplementedError("only causal or all-False masks supported")


def run_cores(in_maps, causal=True, reps=1, **kwargs):
    from concourse import bass_utils

    nc = build_nc(causal, reps)
    return bass_utils.run_bass_kernel_spmd(
        nc, in_maps, core_ids=list(range(N_CORES)), **kwargs
    )


def kernel(Q, K, V, mask):
    kind = _classify_mask(mask)
    in_maps = prep_inputs(Q, K, V)
    res = run_cores(in_maps, causal=(kind == "causal"))
    out = np.concatenate([r["o"] for r in res.results], axis=0)
    return out.reshape(B, H, S, D).astype(np.float32)


if __name__ == "__main__":
    rng = np.random.default_rng(0)
    Q = rng.standard_normal((B, H, S, D), dtype=np.float32)
    K = rng.standard_normal((B, H, S, D), dtype=np.float32)
    V = rng.standard_normal((B, H, S, D), dtype=np.float32)
    mask = np.triu(np.ones((S, S), dtype=bool), k=1)[None, None]
    out = kernel(Q, K, V, mask)
    print("out", out.shape, out.dtype)



# revision 11
# speedup vs baseline: 2.3538x; 2.3538x over previous
"""Causal multi-head attention kernel for Trainium2 (8 NeuronCores), v3.

Problem: B=2, H=16, S=2048, D=64 causal attention (softmax over last axis).
Sharding: 32 (batch, head) pairs split 4-per-core across 8 cores; each core
computes its heads independently (no collectives).

v3 design (engine-balance oriented; HW shows the PE often runs at the cold
1.2 GHz p-state for duty-cycled matmul streams, and ACT costs ~300ns per
ACTIVATE on top of N/1.2GHz):
  - Head PAIRS advance through k-blocks together.  The two heads' QK matmuls
    (contraction = d = 64) are emitted back-to-back with lhsT base partitions
    0 / 64, so the PE row-tiles them into concurrent array halves (2x).
  - exp is computed per merged piece [128, 2heads, 512] (N=1024/instr) and
    SPLIT between ACT (exact, activation) and DVE (Schraudolph one-op bit
    trick: tensor_scalar mult+add -> int16, bitcast bf16) by DVE_EXP share.
  - forward PV (stationary = P^T chunk, rhs = V-block||ones [128,65]) into
    65-wide per-q_sub PSUM accumulators; LDW-bound, p-state immune.
  - intra-block causal mask via GPSIMD tensor multiply (Pool engine).
  - normalize on DVE (reciprocal + per-partition scalar mul), output bf16 in
    [head, r, j, c] layout (q = j*128+r) so out-DMA descriptors are >=512B;
    host inverse-permutes and casts.
  - 2-pass q split per pair (q_subs 0-7 with kb 0-7, then q_subs 8-15 with
    kb 0-15) so PSUM fits: accs 3 banks + st 2x[128,2,512] (4 banks).
"""

import sys

if "/opt/trn_rl_repo" not in sys.path:
    sys.path.insert(0, "/opt/trn_rl_repo")

import numpy as np
import ml_dtypes

B, H, S, D = 2, 16, 2048, 64
N_CORES = 8
HEADS_PER_CORE = (B * H) // N_CORES  # 4
KB = S // 128  # 16 k-blocks per head

_BF16 = ml_dtypes.bfloat16

_built = {}

MODE = "full"
_MODE_FLAGS = {
    "full": "dqepn",
    "qk_noio": "q",
    "exp_pure": "e",
    "qe_pure": "qe",
    "qep_pure": "qep",
    "no_out": "dqep",
}

# Fraction of exp columns computed on DVE (Schraudolph) instead of ACT.
DVE_EXP = 0.4
NORM_BATCH = True  # batched (1 recip + 1 bcast-mul per bank) vs per-q_sub

# Schraudolph constants: exp(s*0.125) ~= bf16_bits(round(s*A + B))
_SCH_A = 0.125 * 1.4426950408889634 * 128.0
_SCH_B = 16256.0 - 4.8


def _pass_plan(causal, pass_qs):
    """For one pass: list of (kb, pieces) where pieces are (qlo, qhi) column
    ranges (multiples of 128) covering q in [pass_qs[0]*128, pass_qs[-1]*128+128)
    with causal q >= 128*kb, split into <=512-wide pieces."""
    q_lo = pass_qs[0] * 128
    q_hi = pass_qs[-1] * 128 + 128
    plan = []
    for kb in range(KB):
        # align piece starts down to 512 so every piece is a contiguous
        # [128, 2, 512] tile (sub-diagonal columns are computed+exp'd but
        # never consumed by PV)
        start = max(q_lo, (128 * kb) // 512 * 512) if causal else q_lo
        if causal and 128 * kb >= q_hi:
            continue
        if start >= q_hi:
            continue
        pieces = []
        c = start
        while c < q_hi:
            ce = min(c + 512, q_hi)
            pieces.append((c, ce))
            c = ce
        plan.append((kb, pieces))
    return plan


def _emit(tc, nc, mybir, qt, kt, vg, tri, o2, causal, reps=1):
    from contextlib import ExitStack, nullcontext

    flags = _MODE_FLAGS[MODE]
    f32 = mybir.dt.float32
    bf = mybir.dt.bfloat16
    i16 = mybir.dt.int16
    Exp = mybir.ActivationFunctionType.Exp
    Mult = mybir.AluOpType.mult
    Add = mybir.AluOpType.add

    with ExitStack() as ctx:
        const = ctx.enter_context(tc.tile_pool(name="const", bufs=1))
        qk = ctx.enter_context(tc.tile_pool(name="qk", bufs=2))
        vpool = ctx.enter_context(tc.tile_pool(name="vp", bufs=2))
        ptp = ctx.enter_context(tc.tile_pool(name="ptp", bufs=4))
        outp = ctx.enter_context(tc.tile_pool(name="outp", bufs=6))
        small = ctx.enter_context(tc.tile_pool(name="small", bufs=6))
        stp = ctx.enter_context(tc.tile_pool(name="stp", bufs=3, space="PSUM"))
        accp = ctx.enter_context(tc.tile_pool(name="accp", bufs=1, space="PSUM"))

        tri_t = const.tile([128, 128], bf, name="tri_t")
        nc.sync.dma_start(tri_t[:, :], tri[:, :])

        warm = const.tile([128, 1], f32, name="warm")
        nc.vector.memset(warm[:, :], 0.0)
        nc.scalar.activation(warm[:, :], warm[:, :], Exp)

        if "d" not in flags:
            qt_c = const.tile([128, S], bf, name="qt_const")
            kt_c = const.tile([128, S], bf, name="kt_const")
            vg_c = const.tile([128, KB, 65], bf, name="vg_const")
            nc.vector.memset(qt_c[:, :], 0.0)
            nc.vector.memset(kt_c[:, :], 0.0)
            nc.vector.memset(vg_c[:, :, :], 0.0)
        if "q" not in flags and "e" in flags:
            cpsum = ctx.enter_context(
                tc.tile_pool(name="cpsum", bufs=1, space="PSUM"))
            st_cA = cpsum.tile([128, 2, 512], f32, name="st_constA")
            st_cB = cpsum.tile([128, 2, 512], f32, name="st_constB")
            nc.vector.memset(st_cA[:, :, :], 0.0)
            nc.vector.memset(st_cB[:, :, :], 0.0)
            st_consts = [st_cA, st_cB]
            st_ctr = [0]

        # DVE/ACT exp assignment: DVE only on odd piece indices so the two
        # engines always read different PSUM bank-pairs (st slot parity)
        dve_acc = [0.0]

        def use_dve():
            if DVE_EXP <= 0.0 or "e" not in flags:
                return False
            dve_acc[0] += DVE_EXP
            if dve_acc[0] >= 1.0:
                dve_acc[0] -= 1.0
                return True
            return False

        with (tc.For_i(0, reps, 1) if reps > 1 else nullcontext()):
          rep = 0
          tiles = {}
          for p in range(HEADS_PER_CORE // 2):
              # all input loads at the top of the rep on the SP queue (kept
              # free of output DMAs so the next rep's loads prefetch early);
              # chunked so pair0's first QK unblocks quickly
              if "d" in flags:
                  qt_t = qk.tile([128, S], bf, tag="qt", name=f"qt_{p}")
                  kt_t = qk.tile([128, S], bf, tag="kt", name=f"kt_{p}")
                  vg_t0 = vpool.tile([128, KB, 65], bf, tag="vg0",
                                     name=f"vg_{p}_0")
                  vg_t1 = vpool.tile([128, KB, 65], bf, tag="vg1",
                                     name=f"vg_{p}_1")
                  nc.sync.dma_start(kt_t[:, :128], kt[p][:, :128])
                  nc.sync.dma_start(qt_t[:, :512], qt[p][:, :512])
                  nc.sync.dma_start(vg_t0[:, :, :], vg[2 * p])
                  nc.sync.dma_start(vg_t1[:, :, :], vg[2 * p + 1])
                  nc.sync.dma_start(kt_t[:, 128:], kt[p][:, 128:])
                  nc.sync.dma_start(qt_t[:, 512:1024], qt[p][:, 512:1024])
                  nc.sync.dma_start(qt_t[:, 1024:], qt[p][:, 1024:])
              else:
                  qt_t, kt_t = qt_c, kt_c
                  vg_t0 = vg_t1 = vg_c
              tiles[p] = (qt_t, kt_t, (vg_t0, vg_t1))
          for p in range(HEADS_PER_CORE // 2):
              qt_t, kt_t, vg_ts = tiles[p]

              for pa, pass_qs in enumerate([list(range(4 * i, 4 * i + 4))
                                            for i in range(4)]):
                  plan = _pass_plan(causal, pass_qs)
                  nsub = len(pass_qs)
                  # accs: flat (h, qs_idx) -> 65-wide accumulator; 7 per bank
                  nacc = 2 * nsub
                  bank_of = lambda fl: fl // nsub
                  accs = [accp.tile([128, nsub, 65], f32,
                                    tag=f"acc{b}", name=f"acc_{p}_{pa}_{b}")
                          for b in range(2)]

                  def acc(h, qi):
                      return accs[h][:, qi, :]

                  # prepass: per-bank first/last PV matmul (emission order)
                  pv_seq = []  # (kb, piece_idx, h, qi)
                  for kb, pieces in plan:
                      for pi, (qlo, qhi) in enumerate(pieces):
                          for h in range(2):
                              qs_list = []
                              for q_sub in range(qlo // 128, qhi // 128):
                                  if causal and q_sub < kb:
                                      continue
                                  qs_list.append(q_sub)
                              if causal and kb > 0 and qs_list \
                                      and qs_list[0] == kb:
                                  qs_list = qs_list[1:] + [kb]
                              for q_sub in qs_list:
                                  pv_seq.append(
                                      (kb, pi, h, q_sub - pass_qs[0]))
                  first_in_bank = {}
                  last_in_bank = {}
                  for idx, (kb, pi, h, qi) in enumerate(pv_seq):
                      b = bank_of(h * nsub + qi)
                      first_in_bank.setdefault(b, idx)
                      last_in_bank[b] = idx
                  first_set = set(first_in_bank.values())
                  last_set = set(last_in_bank.values())

                  # bank -> kb at which it completes (for normalize timing)
                  bank_done_at = {}
                  for idx, (kb, pi, h, qi) in enumerate(pv_seq):
                      if idx in last_set:
                          bank_done_at.setdefault(kb, []).append(
                              bank_of(h * nsub + qi))

                  def emit_qk(kb, pieces):
                      sts = []
                      for (qlo, qhi) in pieces:
                          if "q" not in flags:
                              if "e" in flags:
                                  st_ctr[0] += 1
                                  sts.append(st_consts[st_ctr[0] % 2])
                              else:
                                  sts.append(None)
                              continue
                          st = stp.tile([128, 2, 512], f32, tag="st",
                                        name=f"st_{p}_{pa}_{kb}_{qlo}")
                          w = qhi - qlo
                          for h in range(2):
                              po = 64 * h
                              nc.tensor.matmul(
                                  st[:, h, :w],
                                  lhsT=kt_t[po:po + 64,
                                            kb * 128:(kb + 1) * 128],
                                  rhs=qt_t[po:po + 64, qlo:qhi],
                                  start=True, stop=True,
                              )
                          sts.append(st)
                      return sts

                  def normalize_banks(banks):
                      if "n" not in flags:
                          return
                      for b in banks:
                          fls = [fl for fl in range(nacc) if bank_of(fl) == b]
                          # group by head: contiguous q ranges per head
                          for h in range(2):
                              qis = sorted(fl - h * nsub for fl in fls
                                           if fl // nsub == h)
                              if not qis:
                                  continue
                              qs_lo = pass_qs[0] + qis[0]
                              n = len(qis)
                              ot = outp.tile([128, n, 64], bf,
                                             tag=f"ot{b}_{h}",
                                             name=f"ot_{p}_{pa}_{b}_{h}")
                              if NORM_BATCH:
                                  # one reciprocal over the bank's rowsum
                                  # column + one broadcast multiply
                                  at = accs[b]
                                  j0 = qis[0]
                                  rs = small.tile([128, n], f32,
                                                  tag=f"rs{b}_{h}",
                                                  name=f"rs_{p}_{pa}_{b}_{h}")
                                  nc.vector.reciprocal(
                                      rs[:, :], at[:, j0:j0 + n, 64])
                                  nc.vector.tensor_tensor(
                                      ot[:, :, :], at[:, j0:j0 + n, :64],
                                      rs[:, :].unsqueeze(2).broadcast_to(
                                          [128, n, 64]),
                                      mybir.AluOpType.mult)
                              else:
                                  for j, qi in enumerate(qis):
                                      a = acc(h, qi)
                                      rs = small.tile(
                                          [128, 1], f32, tag="rs",
                                          name=f"rs_{p}_{pa}_{b}_{h}_{j}")
                                      nc.vector.reciprocal(rs[:, :],
                                                           a[:, 64:65])
                                      nc.vector.tensor_scalar_mul(
                                          ot[:, j, :], a[:, :64], rs[:, :])
                              h_g = 2 * p + h
                              nc.gpsimd.dma_start(
                                  o2[h_g, :, qs_lo:qs_lo + n, :],
                                  ot[:, :, :])

                  # two strips of QK in flight ahead of exp/PV
                  sts_q = [emit_qk(*plan[0])]
                  if len(plan) > 1:
                      sts_q.append(emit_qk(*plan[1]))
                  for step, (kb, pieces) in enumerate(plan):
                      sts = sts_q.pop(0)
                      q0 = 128 * kb if causal else 0
                      # exp (ACT or DVE per piece, merged across the 2 heads)
                      pts = []
                      for (qlo, qhi), st in zip(pieces, sts):
                          if "e" not in flags:
                              pts.append(None)
                              continue
                          w = qhi - qlo
                          pt = ptp.tile([128, 2, 512], bf, tag="pt",
                                        name=f"pt_{p}_{pa}_{kb}_{qlo}")
                          if use_dve():
                              nc.vector.tensor_scalar(
                                  pt[:, :, :w].bitcast(i16), st[:, :, :w],
                                  _SCH_A, _SCH_B, Mult, Add)
                          else:
                              nc.scalar.activation(pt[:, :, :w], st[:, :, :w],
                                                   Exp, scale=0.125)
                          if causal and qlo <= q0 < qhi:
                              # intra-block mask on the diagonal 128 cols
                              dg = q0 - qlo
                              for h in range(2):
                                  nc.gpsimd.tensor_mul(pt[:, h, dg:dg + 128],
                                                       pt[:, h, dg:dg + 128],
                                                       tri_t[:, :])
                          pts.append(pt)
                      # QK two strips ahead keeps PE busy under exp without
                      # blocking on the current strip's exp (3 st slots)
                      if step + 2 < len(plan):
                          sts_q.append(emit_qk(*plan[step + 2]))
                      # forward PV for strip kb
                      if "p" in flags and "e" in flags:
                          for idx, (kb2, pi, h, qi) in enumerate(pv_seq):
                              if kb2 != kb:
                                  continue
                              qlo, qhi = pieces[pi]
                              pt = pts[pi]
                              q_sub = pass_qs[0] + qi
                              m = q_sub * 128 - qlo
                              nc.tensor.matmul(
                                  acc(h, qi),
                                  lhsT=pt[:, h, m:m + 128],
                                  rhs=vg_ts[h][:, kb, :],
                                  start=(idx in first_set),
                                  stop=(idx in last_set),
                              )
                          normalize_banks(bank_done_at.get(kb, []))


def build_nc(causal=True, reps=1):
    key = ("nc3", causal, reps, MODE, DVE_EXP, NORM_BATCH)
    if key in _built:
        return _built[key]
    import concourse.bacc as bacc
    from concourse import mybir, tile

    nc = bacc.Bacc("TRN2", target_bir_lowering=False, debug=False,
                   num_devices=N_CORES)
    qt = nc.dram_tensor("qt", (HEADS_PER_CORE // 2, 128, S),
                        mybir.dt.bfloat16, kind="ExternalInput").ap()
    kt = nc.dram_tensor("kt", (HEADS_PER_CORE // 2, 128, S),
                        mybir.dt.bfloat16, kind="ExternalInput").ap()
    vg = nc.dram_tensor("vg", (HEADS_PER_CORE, 128, KB, 65),
                        mybir.dt.bfloat16, kind="ExternalInput").ap()
    tri = nc.dram_tensor("tri", (128, 128), mybir.dt.bfloat16,
                         kind="ExternalInput").ap()
    # output in [head, r, j, c] layout, q = j*128 + r (big DMA descriptors)
    o2 = nc.dram_tensor("o2", (HEADS_PER_CORE, 128, KB, D), mybir.dt.bfloat16,
                        kind="ExternalOutput").ap()
    with tile.TileContext(nc) as tc:
        _emit(tc, nc, mybir, qt, kt, vg, tri, o2, causal, reps)
    nc.compile()
    _built[key] = nc
    return nc


def prep_inputs(Q, K, V):
    Qf = np.ascontiguousarray(Q, dtype=np.float32).reshape(B * H, S, D)
    Kf = np.ascontiguousarray(K, dtype=np.float32).reshape(B * H, S, D)
    Vf = np.ascontiguousarray(V, dtype=np.float32).reshape(B * H, S, D)

    Qt = np.ascontiguousarray(Qf.transpose(0, 2, 1)).astype(_BF16)
    Kt = np.ascontiguousarray(Kf.transpose(0, 2, 1)).astype(_BF16)

    Vb = Vf.astype(_BF16)
    vg_all = np.empty((B * H, 128, KB, 65), dtype=_BF16)
    vg_all[:, :, :, :64] = Vb.reshape(B * H, KB, 128, D).transpose(0, 2, 1, 3)
    vg_all[:, :, :, 64] = _BF16(1.0)

    tri_np = (np.tril(np.ones((128, 128), dtype=np.float32))
              .T.astype(_BF16))
    tri_np = np.ascontiguousarray(tri_np)

    in_maps = []
    for c in range(N_CORES):
        h0 = c * HEADS_PER_CORE
        qt_c = np.empty((HEADS_PER_CORE // 2, 128, S), dtype=_BF16)
        kt_c = np.empty((HEADS_PER_CORE // 2, 128, S), dtype=_BF16)
        for p in range(HEADS_PER_CORE // 2):
            qt_c[p, :64] = Qt[h0 + 2 * p]
            qt_c[p, 64:] = Qt[h0 + 2 * p + 1]
            kt_c[p, :64] = Kt[h0 + 2 * p]
            kt_c[p, 64:] = Kt[h0 + 2 * p + 1]
        in_maps.append({
            "qt": qt_c,
            "kt": kt_c,
            "vg": np.ascontiguousarray(vg_all[h0:h0 + HEADS_PER_CORE]),
            "tri": tri_np,
        })
    return in_maps


def _classify_mask(mask):
    m = np.asarray(mask).reshape(S, S)
    if not m.any():
        return "dense"
    if np.array_equal(m, np.triu(np.ones((S, S), dtype=bool), k=1)):
        return "causal"
    raise NotImplementedError("only causal or all-False masks supported")


def run_cores(in_maps, causal=True, reps=1, **kwargs):
    from concourse import bass_utils

    nc = build_nc(causal, reps)
    return bass_utils.run_bass_kernel_spmd(
        nc, in_maps, core_ids=list(range(N_CORES)), **kwargs
    )


def kernel(Q, K, V, mask):
    kind = _classify_mask(mask)
    in_maps = prep_inputs(Q, K, V)
    res = run_cores(in_maps, causal=(kind == "causal"))
    outs = []
    for r in res.results:
        o2 = np.asarray(r["o2"], dtype=np.float32)  # [4, 128, 16, 64]
        o = o2.transpose(0, 2, 1, 3).reshape(HEADS_PER_CORE, S, D)
        outs.append(o)
    out = np.concatenate(outs, axis=0)
    return np.ascontiguousarray(out.reshape(B, H, S, D), dtype=np.float32)


if __name__ == "__main__":
    rng = np.random.default_rng(0)
    Q = rng.standard_normal((B, H, S, D), dtype=np.float32)
    K = rng.standard_normal((B, H, S, D), dtype=np.float32)
    V = rng.standard_normal((B, H, S, D), dtype=np.float32)
    mask = np.triu(np.ones((S, S), dtype=bool), k=1)[None, None]
    out = kernel(Q, K, V, mask)
    print("out", out.shape, out.dtype)


# revision 14
# speedup vs baseline: 2.4437x; 1.0382x over previous
"""Causal multi-head attention kernel for Trainium2 (8 NeuronCores), v3.

Problem: B=2, H=16, S=2048, D=64 causal attention (softmax over last axis).
Sharding: 32 (batch, head) pairs split 4-per-core across 8 cores; each core
computes its heads independently (no collectives).

v3 design (engine-balance oriented; HW shows the PE often runs at the cold
1.2 GHz p-state for duty-cycled matmul streams, and ACT costs ~300ns per
ACTIVATE on top of N/1.2GHz):
  - Head PAIRS advance through k-blocks together.  The two heads' QK matmuls
    (contraction = d = 64) are emitted back-to-back with lhsT base partitions
    0 / 64, so the PE row-tiles them into concurrent array halves (2x).
  - exp is computed per merged piece [128, 2heads, 512] (N=1024/instr) and
    SPLIT between ACT (exact, activation) and DVE (Schraudolph one-op bit
    trick: tensor_scalar mult+add -> int16, bitcast bf16) by DVE_EXP share.
  - forward PV (stationary = P^T chunk, rhs = V-block||ones [128,65]) into
    65-wide per-q_sub PSUM accumulators; LDW-bound, p-state immune.
  - intra-block causal mask via GPSIMD tensor multiply (Pool engine).
  - normalize on DVE (reciprocal + per-partition scalar mul), output bf16 in
    [head, r, j, c] layout (q = j*128+r) so out-DMA descriptors are >=512B;
    host inverse-permutes and casts.
  - 2-pass q split per pair (q_subs 0-7 with kb 0-7, then q_subs 8-15 with
    kb 0-15) so PSUM fits: accs 3 banks + st 2x[128,2,512] (4 banks).
"""

import sys

if "/opt/trn_rl_repo" not in sys.path:
    sys.path.insert(0, "/opt/trn_rl_repo")

import numpy as np
import ml_dtypes

B, H, S, D = 2, 16, 2048, 64
N_CORES = 8
HEADS_PER_CORE = (B * H) // N_CORES  # 4
KB = S // 128  # 16 k-blocks per head

_BF16 = ml_dtypes.bfloat16

_built = {}

MODE = "full"
_MODE_FLAGS = {
    "full": "dqepn",
    "qk_noio": "q",
    "exp_pure": "e",
    "qe_pure": "qe",
    "qep_pure": "qep",
    "no_out": "dqep",
}

# Fraction of exp columns computed on DVE (Schraudolph) instead of ACT.
DVE_EXP = 0.4
NORM_BATCH = True  # batched (1 recip + 1 bcast-mul per bank) vs per-q_sub

# Schraudolph constants: exp(s*0.125) ~= bf16_bits(round(s*A + B))
_SCH_A = 0.125 * 1.4426950408889634 * 128.0
_SCH_B = 16256.0 - 4.8


def _pass_plan(causal, pass_qs):
    """For one pass: list of (kb, pieces) where pieces are (qlo, qhi) column
    ranges (multiples of 128) covering q in [pass_qs[0]*128, pass_qs[-1]*128+128)
    with causal q >= 128*kb, split into <=512-wide pieces."""
    q_lo = pass_qs[0] * 128
    q_hi = pass_qs[-1] * 128 + 128
    plan = []
    for kb in range(KB):
        # align piece starts down to 512 so every piece is a contiguous
        # [128, 2, 512] tile (sub-diagonal columns are computed+exp'd but
        # never consumed by PV)
        start = max(q_lo, (128 * kb) // 512 * 512) if causal else q_lo
        if causal and 128 * kb >= q_hi:
            continue
        if start >= q_hi:
            continue
        pieces = []
        c = start
        while c < q_hi:
            ce = min(c + 512, q_hi)
            pieces.append((c, ce))
            c = ce
        plan.append((kb, pieces))
    return plan


def _emit(tc, nc, mybir, qt, kt, vg, tri, o2, causal, reps=1):
    from contextlib import ExitStack, nullcontext

    flags = _MODE_FLAGS[MODE]
    f32 = mybir.dt.float32
    bf = mybir.dt.bfloat16
    i16 = mybir.dt.int16
    Exp = mybir.ActivationFunctionType.Exp
    Mult = mybir.AluOpType.mult
    Add = mybir.AluOpType.add

    with ExitStack() as ctx:
        const = ctx.enter_context(tc.tile_pool(name="const", bufs=1))
        qk = ctx.enter_context(tc.tile_pool(name="qk", bufs=3))
        vpool = ctx.enter_context(tc.tile_pool(name="vp", bufs=3))
        ptp = ctx.enter_context(tc.tile_pool(name="ptp", bufs=4))
        outp = ctx.enter_context(tc.tile_pool(name="outp", bufs=6))
        small = ctx.enter_context(tc.tile_pool(name="small", bufs=6))
        stp = ctx.enter_context(tc.tile_pool(name="stp", bufs=3, space="PSUM"))
        accp = ctx.enter_context(tc.tile_pool(name="accp", bufs=1, space="PSUM"))

        tri_t = const.tile([128, 128], bf, name="tri_t")
        nc.sync.dma_start(tri_t[:, :], tri[:, :])

        warm = const.tile([128, 1], f32, name="warm")
        nc.vector.memset(warm[:, :], 0.0)
        nc.scalar.activation(warm[:, :], warm[:, :], Exp)

        if "d" not in flags:
            qt_c = const.tile([128, S], bf, name="qt_const")
            kt_c = const.tile([128, S], bf, name="kt_const")
            vg_c = const.tile([128, KB, 65], bf, name="vg_const")
            nc.vector.memset(qt_c[:, :], 0.0)
            nc.vector.memset(kt_c[:, :], 0.0)
            nc.vector.memset(vg_c[:, :, :], 0.0)
        if "q" not in flags and "e" in flags:
            cpsum = ctx.enter_context(
                tc.tile_pool(name="cpsum", bufs=1, space="PSUM"))
            st_cA = cpsum.tile([128, 2, 512], f32, name="st_constA")
            st_cB = cpsum.tile([128, 2, 512], f32, name="st_constB")
            nc.vector.memset(st_cA[:, :, :], 0.0)
            nc.vector.memset(st_cB[:, :, :], 0.0)
            st_consts = [st_cA, st_cB]
            st_ctr = [0]

        # DVE/ACT exp assignment: DVE only on odd piece indices so the two
        # engines always read different PSUM bank-pairs (st slot parity)
        dve_acc = [0.0]

        def use_dve():
            if DVE_EXP <= 0.0 or "e" not in flags:
                return False
            dve_acc[0] += DVE_EXP
            if dve_acc[0] >= 1.0:
                dve_acc[0] -= 1.0
                return True
            return False

        with (tc.For_i(0, reps, 1) if reps > 1 else nullcontext()):
          rep = 0
          tiles = {}
          for p in range(HEADS_PER_CORE // 2):
              # all input loads at the top of the rep on the SP queue (kept
              # free of output DMAs so the next rep's loads prefetch early);
              # chunked so pair0's first QK unblocks quickly
              if "d" in flags:
                  qt_t = qk.tile([128, S], bf, tag="qt", name=f"qt_{p}")
                  kt_t = qk.tile([128, S], bf, tag="kt", name=f"kt_{p}")
                  vg_t0 = vpool.tile([128, KB, 65], bf, tag="vg0",
                                     name=f"vg_{p}_0")
                  vg_t1 = vpool.tile([128, KB, 65], bf, tag="vg1",
                                     name=f"vg_{p}_1")
                  nc.sync.dma_start(kt_t[:, :128], kt[p][:, :128])
                  nc.sync.dma_start(qt_t[:, :512], qt[p][:, :512])
                  nc.sync.dma_start(vg_t0[:, :, :], vg[2 * p])
                  nc.sync.dma_start(vg_t1[:, :, :], vg[2 * p + 1])
                  nc.sync.dma_start(kt_t[:, 128:], kt[p][:, 128:])
                  nc.sync.dma_start(qt_t[:, 512:1024], qt[p][:, 512:1024])
                  nc.sync.dma_start(qt_t[:, 1024:], qt[p][:, 1024:])
              else:
                  qt_t, kt_t = qt_c, kt_c
                  vg_t0 = vg_t1 = vg_c
              tiles[p] = (qt_t, kt_t, (vg_t0, vg_t1))
          for p in range(HEADS_PER_CORE // 2):
              qt_t, kt_t, vg_ts = tiles[p]

              for pa, pass_qs in enumerate([list(range(4 * i, 4 * i + 4))
                                            for i in range(4)]):
                  plan = _pass_plan(causal, pass_qs)
                  nsub = len(pass_qs)
                  # accs: flat (h, qs_idx) -> 65-wide accumulator; 7 per bank
                  nacc = 2 * nsub
                  bank_of = lambda fl: fl // nsub
                  accs = [accp.tile([128, nsub, 65], f32,
                                    tag=f"acc{b}", name=f"acc_{p}_{pa}_{b}")
                          for b in range(2)]

                  def acc(h, qi):
                      return accs[h][:, qi, :]

                  # prepass: per-bank first/last PV matmul (emission order)
                  pv_seq = []  # (kb, piece_idx, h, qi)
                  for kb, pieces in plan:
                      for pi, (qlo, qhi) in enumerate(pieces):
                          for h in range(2):
                              qs_list = []
                              for q_sub in range(qlo // 128, qhi // 128):
                                  if causal and q_sub < kb:
                                      continue
                                  qs_list.append(q_sub)
                              if causal and kb > 0 and qs_list \
                                      and qs_list[0] == kb:
                                  qs_list = qs_list[1:] + [kb]
                              for q_sub in qs_list:
                                  pv_seq.append(
                                      (kb, pi, h, q_sub - pass_qs[0]))
                  first_in_bank = {}
                  last_in_bank = {}
                  for idx, (kb, pi, h, qi) in enumerate(pv_seq):
                      b = bank_of(h * nsub + qi)
                      first_in_bank.setdefault(b, idx)
                      last_in_bank[b] = idx
                  first_set = set(first_in_bank.values())
                  last_set = set(last_in_bank.values())

                  # bank -> kb at which it completes (for normalize timing)
                  bank_done_at = {}
                  for idx, (kb, pi, h, qi) in enumerate(pv_seq):
                      if idx in last_set:
                          bank_done_at.setdefault(kb, []).append(
                              bank_of(h * nsub + qi))

                  def emit_qk(kb, pieces):
                      sts = []
                      for (qlo, qhi) in pieces:
                          if "q" not in flags:
                              if "e" in flags:
                                  st_ctr[0] += 1
                                  sts.append(st_consts[st_ctr[0] % 2])
                              else:
                                  sts.append(None)
                              continue
                          st = stp.tile([128, 2, 512], f32, tag="st",
                                        name=f"st_{p}_{pa}_{kb}_{qlo}")
                          w = qhi - qlo
                          for h in range(2):
                              po = 64 * h
                              nc.tensor.matmul(
                                  st[:, h, :w],
                                  lhsT=kt_t[po:po + 64,
                                            kb * 128:(kb + 1) * 128],
                                  rhs=qt_t[po:po + 64, qlo:qhi],
                                  start=True, stop=True,
                              )
                          sts.append(st)
                      return sts

                  def normalize_banks(banks):
                      if "n" not in flags:
                          return
                      for b in banks:
                          fls = [fl for fl in range(nacc) if bank_of(fl) == b]
                          # group by head: contiguous q ranges per head
                          for h in range(2):
                              qis = sorted(fl - h * nsub for fl in fls
                                           if fl // nsub == h)
                              if not qis:
                                  continue
                              qs_lo = pass_qs[0] + qis[0]
                              n = len(qis)
                              ot = outp.tile([128, n, 64], bf,
                                             tag=f"ot{b}_{h}",
                                             name=f"ot_{p}_{pa}_{b}_{h}")
                              if NORM_BATCH:
                                  # one reciprocal over the bank's rowsum
                                  # column + one broadcast multiply
                                  at = accs[b]
                                  j0 = qis[0]
                                  rs = small.tile([128, n], f32,
                                                  tag=f"rs{b}_{h}",
                                                  name=f"rs_{p}_{pa}_{b}_{h}")
                                  nc.vector.reciprocal(
                                      rs[:, :], at[:, j0:j0 + n, 64])
                                  nc.vector.tensor_tensor(
                                      ot[:, :, :], at[:, j0:j0 + n, :64],
                                      rs[:, :].unsqueeze(2).broadcast_to(
                                          [128, n, 64]),
                                      mybir.AluOpType.mult)
                              else:
                                  for j, qi in enumerate(qis):
                                      a = acc(h, qi)
                                      rs = small.tile(
                                          [128, 1], f32, tag="rs",
                                          name=f"rs_{p}_{pa}_{b}_{h}_{j}")
                                      nc.vector.reciprocal(rs[:, :],
                                                           a[:, 64:65])
                                      nc.vector.tensor_scalar_mul(
                                          ot[:, j, :], a[:, :64], rs[:, :])
                              h_g = 2 * p + h
                              nc.gpsimd.dma_start(
                                  o2[h_g, :, qs_lo:qs_lo + n, :],
                                  ot[:, :, :])

                  # two strips of QK in flight ahead of exp/PV
                  sts_q = [emit_qk(*plan[0])]
                  if len(plan) > 1:
                      sts_q.append(emit_qk(*plan[1]))
                  for step, (kb, pieces) in enumerate(plan):
                      sts = sts_q.pop(0)
                      q0 = 128 * kb if causal else 0
                      # exp (ACT or DVE per piece, merged across the 2 heads)
                      pts = []
                      for (qlo, qhi), st in zip(pieces, sts):
                          if "e" not in flags:
                              pts.append(None)
                              continue
                          w = qhi - qlo
                          pt = ptp.tile([128, 2, 512], bf, tag="pt",
                                        name=f"pt_{p}_{pa}_{kb}_{qlo}")
                          if use_dve():
                              nc.vector.tensor_scalar(
                                  pt[:, :, :w].bitcast(i16), st[:, :, :w],
                                  _SCH_A, _SCH_B, Mult, Add)
                          else:
                              nc.scalar.activation(pt[:, :, :w], st[:, :, :w],
                                                   Exp, scale=0.125)
                          if causal and qlo <= q0 < qhi:
                              # intra-block mask on the diagonal 128 cols
                              dg = q0 - qlo
                              for h in range(2):
                                  nc.gpsimd.tensor_mul(pt[:, h, dg:dg + 128],
                                                       pt[:, h, dg:dg + 128],
                                                       tri_t[:, :])
                          pts.append(pt)
                      # QK two strips ahead keeps PE busy under exp without
                      # blocking on the current strip's exp (3 st slots)
                      if step + 2 < len(plan):
                          sts_q.append(emit_qk(*plan[step + 2]))
                      # forward PV for strip kb
                      if "p" in flags and "e" in flags:
                          for idx, (kb2, pi, h, qi) in enumerate(pv_seq):
                              if kb2 != kb:
                                  continue
                              qlo, qhi = pieces[pi]
                              pt = pts[pi]
                              q_sub = pass_qs[0] + qi
                              m = q_sub * 128 - qlo
                              nc.tensor.matmul(
                                  acc(h, qi),
                                  lhsT=pt[:, h, m:m + 128],
                                  rhs=vg_ts[h][:, kb, :],
                                  start=(idx in first_set),
                                  stop=(idx in last_set),
                              )
                          normalize_banks(bank_done_at.get(kb, []))


def build_nc(causal=True, reps=1):
    key = ("nc3", causal, reps, MODE, DVE_EXP, NORM_BATCH)
    if key in _built:
        return _built[key]
    import concourse.bacc as bacc
    from concourse import mybir, tile

    nc = bacc.Bacc("TRN2", target_bir_lowering=False, debug=False,
                   num_devices=N_CORES)
    qt = nc.dram_tensor("qt", (HEADS_PER_CORE // 2, 128, S),
                        mybir.dt.bfloat16, kind="ExternalInput").ap()
    kt = nc.dram_tensor("kt", (HEADS_PER_CORE // 2, 128, S),
                        mybir.dt.bfloat16, kind="ExternalInput").ap()
    vg = nc.dram_tensor("vg", (HEADS_PER_CORE, 128, KB, 65),
                        mybir.dt.bfloat16, kind="ExternalInput").ap()
    tri = nc.dram_tensor("tri", (128, 128), mybir.dt.bfloat16,
                         kind="ExternalInput").ap()
    # output in [head, r, j, c] layout, q = j*128 + r (big DMA descriptors)
    o2 = nc.dram_tensor("o2", (HEADS_PER_CORE, 128, KB, D), mybir.dt.bfloat16,
                        kind="ExternalOutput").ap()
    with tile.TileContext(nc) as tc:
        _emit(tc, nc, mybir, qt, kt, vg, tri, o2, causal, reps)
    nc.compile()
    _built[key] = nc
    return nc


def prep_inputs(Q, K, V):
    Qf = np.ascontiguousarray(Q, dtype=np.float32).reshape(B * H, S, D)
    Kf = np.ascontiguousarray(K, dtype=np.float32).reshape(B * H, S, D)
    Vf = np.ascontiguousarray(V, dtype=np.float32).reshape(B * H, S, D)

    Qt = np.ascontiguousarray(Qf.transpose(0, 2, 1)).astype(_BF16)
    Kt = np.ascontiguousarray(Kf.transpose(0, 2, 1)).astype(_BF16)

    Vb = Vf.astype(_BF16)
    vg_all = np.empty((B * H, 128, KB, 65), dtype=_BF16)
    vg_all[:, :, :, :64] = Vb.reshape(B * H, KB, 128, D).transpose(0, 2, 1, 3)
    vg_all[:, :, :, 64] = _BF16(1.0)

    tri_np = (np.tril(np.ones((128, 128), dtype=np.float32))
              .T.astype(_BF16))
    tri_np = np.ascontiguousarray(tri_np)

    in_maps = []
    for c in range(N_CORES):
        h0 = c * HEADS_PER_CORE
        qt_c = np.empty((HEADS_PER_CORE // 2, 128, S), dtype=_BF16)
        kt_c = np.empty((HEADS_PER_CORE // 2, 128, S), dtype=_BF16)
        for p in range(HEADS_PER_CORE // 2):
            qt_c[p, :64] = Qt[h0 + 2 * p]
            qt_c[p, 64:] = Qt[h0 + 2 * p + 1]
            kt_c[p, :64] = Kt[h0 + 2 * p]
            kt_c[p, 64:] = Kt[h0 + 2 * p + 1]
        in_maps.append({
            "qt": qt_c,
            "kt": kt_c,
            "vg": np.ascontiguousarray(vg_all[h0:h0 + HEADS_PER_CORE]),
            "tri": tri_np,
        })
    return in_maps


def _classify_mask(mask):
    m = np.asarray(mask).reshape(S, S)
    if not m.any():
        return "dense"
    if np.array_equal(m, np.triu(np.ones((S, S), dtype=bool), k=1)):
        return "causal"
    raise NotImplementedError("only causal or all-False masks supported")


def run_cores(in_maps, causal=True, reps=1, **kwargs):
    from concourse import bass_utils

    nc = build_nc(causal, reps)
    return bass_utils.run_bass_kernel_spmd(
        nc, in_maps, core_ids=list(range(N_CORES)), **kwargs
    )


def kernel(Q, K, V, mask):
    kind = _classify_mask(mask)
    in_maps = prep_inputs(Q, K, V)
    res = run_cores(in_maps, causal=(kind == "causal"))
    outs = []
    for r in res.results:
        o2 = np.asarray(r["o2"], dtype=np.float32)  # [4, 128, 16, 64]
        o = o2.transpose(0, 2, 1, 3).reshape(HEADS_PER_CORE, S, D)
        outs.append(o)
    out = np.concatenate(outs, axis=0)
    return np.ascontiguousarray(out.reshape(B, H, S, D), dtype=np.float32)


if __name__ == "__main__":
    rng = np.random.default_rng(0)
    Q = rng.standard_normal((B, H, S, D), dtype=np.float32)
    K = rng.standard_normal((B, H, S, D), dtype=np.float32)
    V = rng.standard_normal((B, H, S, D), dtype=np.float32)
    mask = np.triu(np.ones((S, S), dtype=bool), k=1)[None, None]
    out = kernel(Q, K, V, mask)
    print("out", out.shape, out.dtype)
